# revision 6
# baseline (speedup 1.0000x reference)
"""Trainium2 Bass kernel for nn_MMN_7361573945989 (MatchNet corr/attention).

Math (per batch b):
  qn_l = l2norm_c(fq_l); sn_l = l2norm_c(fs_l)           l in {4, 3}
  logits[p, q] = TEMP * (w0 * qn4.T@sn4 + w1 * qn3.T@sn3)[p, q]
  attn = softmax_q(logits)
  att_fq[c, p] = sum_q attn[p, q] * f_s[c, q]
  fq_out = l2norm_c(f_q) + l2norm_c(att_fq) * ATT_WT
  returns (fq_out, att_fq)

Sharding: 8 cores = 2 batches x 4 query-pixel shards of 900.

Per-core kernel (transposed orientation, logits live as [q, p] tiles):
  - channel-norm sums-of-squares via ones-vector matmuls (contract partition dim)
  - row-vector broadcasts across partitions via K=1 matmuls
  - normalization scales folded into the bf16 matmul operands
    (TEMP*w into the support side), so logits accumulate in one PSUM group
  - softmax without max-subtraction: logits = 20*(w.cos) are bounded (|cos|<=1)
  - exp -> SBUF-resident bf16 expT; V transposed on PE; AV + denominator
    matmuls accumulate [c, p] PSUM tiles directly in output orientation.
"""

import sys
from contextlib import ExitStack

import numpy as np

sys.path.insert(0, "/opt/trn_rl_repo")

import concourse.bass as bass  # noqa: E402
import concourse.masks as masks  # noqa: E402
import concourse.tile as tile  # noqa: E402
from concourse import mybir  # noqa: E402
from concourse.bass_utils import run_bass_kernel_spmd  # noqa: E402

B, H, W = 2, 60, 60
HW = H * W  # 3600
C3, C4, CV = 1024, 2048, 512
TEMP = 20.0
ATT_WT = 0.3
NCORES = 8
PSH = 4  # query-pixel shards per batch
P = HW // PSH  # 900 query pixels per core
PB = P // 2  # 450, p-block (one PSUM bank of fp32)
NQC = (HW + 127) // 128  # 29 support-pixel chunks
QT = HW - (NQC - 1) * 128  # 16 rows in the tail chunk
NC4, NC3, NCV = C4 // 128, C3 // 128, CV // 128  # 16, 8, 4
NCI = NC4 + NC3  # 24 combined channel chunks

F32 = mybir.dt.float32
BF16 = mybir.dt.bfloat16
AF = mybir.ActivationFunctionType

_MAX_WAITS_PER_INST = 1


def _patched_drain_and_barrier(self, tick_clock, wait_clock):
    """Tile's kernel-tail drain carries one sem wait per engine/queue; the
    walrus build used here accepts only one sync wait per CTRL instruction.
    Split the waits across extra sync-engine nops."""
    drain_inst = self.nc.sync.drain()
    wait_clock.add_sem_waits(
        drain_inst.ins, tile.ScopedClock({None: tick_clock.global_clock})
    )
    si = drain_inst.ins.sync_info
    if si is not None and len(si.on_wait) > _MAX_WAITS_PER_INST:
        waits = list(si.on_wait)
        drain_inst.ins.sync_info = mybir.SyncInfo(
            on_wait=waits[:_MAX_WAITS_PER_INST], on_update=list(si.on_update)
        )
        for i in range(_MAX_WAITS_PER_INST, len(waits), _MAX_WAITS_PER_INST):
            nop = self.nc.sync.nop()
            nop.ins.sync_info = mybir.SyncInfo(
                on_wait=waits[i : i + _MAX_WAITS_PER_INST], on_update=[]
            )
    self.nc.all_engine_barrier()
    assert self.sems is not None
    popped = self.nc._tile_sem_poison_stack.pop()
    assert popped is self._sem_poison
    self.nc.clear_and_free_semaphores(list(self.sems.allocated().values()))
    self.nc.all_engine_barrier()


tile.TileContext._drain_and_barrier = _patched_drain_and_barrier


def _split_sync_waits(nc, max_waits=_MAX_WAITS_PER_INST):
    """Walrus here accepts at most one sync wait per instruction; move excess
    waits onto same-engine nops inserted immediately before the instruction."""
    ctr = 0
    for f in nc.m.functions:
        for blk in f.blocks:
            insts = list(blk.instructions)
            out = []
            changed = False
            for inst in insts:
                si = inst.sync_info
                if si is not None and len(si.on_wait) > max_waits:
                    waits = list(si.on_wait)
                    for i0 in range(max_waits, len(waits), max_waits):
                        ctr += 1
                        nop = mybir.InstNoOp(
                            name=f"waitsplit-{ctr}",
                            engine=inst.engine,
                            bass_nofuse=True,
                            sync_info=mybir.SyncInfo(
                                on_wait=waits[i0 : i0 + max_waits], on_update=[]
                            ),
                        )
                        nc.register_instruction(nop, overwrite=True)
                        out.append(nop)
                    inst.sync_info = mybir.SyncInfo(
                        on_wait=waits[:max_waits], on_update=list(si.on_update)
                    )
                    changed = True
                out.append(inst)
            if changed:
                blk.instructions = out


def build():
    nc = bass.Bass()
    q4 = nc.dram_tensor("q4", [C4, P], F32, kind="ExternalInput")
    q3 = nc.dram_tensor("q3", [C3, P], F32, kind="ExternalInput")
    s4 = nc.dram_tensor("s4", [C4, HW], F32, kind="ExternalInput")
    s3 = nc.dram_tensor("s3", [C3, HW], F32, kind="ExternalInput")
    v = nc.dram_tensor("v", [CV, HW], F32, kind="ExternalInput")
    fq = nc.dram_tensor("fq", [CV, P], F32, kind="ExternalInput")
    wv = nc.dram_tensor("wv", [1, 2], F32, kind="ExternalInput")  # [T*w0, T*w1]
    att_o = nc.dram_tensor("att_o", [CV, P], F32, kind="ExternalOutput")
    fq_o = nc.dram_tensor("fq_o", [CV, P], F32, kind="ExternalOutput")

    def load_blocks(dst, dst_cols, ci0, src, col0, ncols, n_ci, group=4):
        """Load `n_ci` row-blocks of 128 from DRAM `src` (cols [col0,col0+ncols))
        into SBUF tile `dst` whose free layout is (ci, dst_cols)."""
        srcr = src[:].rearrange("(ci c) x -> c ci x", c=128)
        dstr = dst[:].rearrange("c (ci x) -> c ci x", x=dst_cols)
        for g0 in range(0, n_ci, group):
            g = min(group, n_ci - g0)
            nc.sync.dma_start(
                dstr[:, ci0 + g0 : ci0 + g0 + g, 0:ncols],
                srcr[:, g0 : g0 + g, col0 : col0 + ncols],
            )

    with tile.TileContext(nc) as tc:
        with ExitStack() as octx:
            cpool = octx.enter_context(tc.tile_pool(name="const", bufs=1))
            ident = cpool.tile([128, 128], F32)
            masks.make_identity(nc, ident[:])
            ones_col = cpool.tile([128, 1], BF16)
            nc.gpsimd.memset(ones_col[:], 1.0)
            ones_row = cpool.tile([1, 128], F32)
            nc.gpsimd.memset(ones_row[:], 1.0)
            w_sb = cpool.tile([1, 2], F32)
            nc.sync.dma_start(w_sb[:], wv[:])

            pers = octx.enter_context(tc.tile_pool(name="pers", bufs=1))
            qns = pers.tile([128, NCI * P], BF16)  # scaled query feats (ci, p)
            fqn = pers.tile([128, NCV * P], F32)  # normalized f_q (ci, p)
            expT = pers.tile([128, NQC * P], BF16)  # exp(logits) (qc; q, p)
            vT = pers.tile([128, NQC * CV], BF16)  # f_s transposed (qc; q, c)
            # zero the tail-chunk region so K=128 matmuls over the tail are
            # exact (rows [0:QT] are overwritten with real data later; memset
            # from partition 16 is unsupported, so clear all 128 partitions)
            nc.gpsimd.memset(expT[:, (NQC - 1) * P : NQC * P], 0.0)
            nc.gpsimd.memset(vT[:, (NQC - 1) * CV : NQC * CV], 0.0)

            # ---------------- prep: query-side normalization ----------------
            with ExitStack() as pctx:
                xpool = pctx.enter_context(tc.tile_pool(name="prepx", bufs=2))
                sqpool = pctx.enter_context(tc.tile_pool(name="prepsq", bufs=2))
                mini = pctx.enter_context(tc.tile_pool(name="prepmini", bufs=2))
                pps = pctx.enter_context(
                    tc.tile_pool(name="prepps", bufs=1, space="PSUM")
                )

                # (src, n_ci, dest tile, dest ci0, dest dtype)
                layers = [
                    (q4, NC4, qns, 0),
                    (q3, NC3, qns, NC4),
                    (fq, NCV, fqn, 0),
                ]
                for li, (src, n_ci, dst, ci0) in enumerate(layers):
                    ss = [
                        pps.tile([1, PB], F32, tag=f"ss{pb}", name=f"ss{pb}") for pb in range(2)
                    ]
                    for g0 in range(0, n_ci, 4):
                        g = min(4, n_ci - g0)
                        if dst is fqn:
                            # load straight into the persistent fp32 tile
                            load_blocks(fqn, P, g0, src, 0, P, g)
                            xg = fqn[:, g0 * P : (g0 + g) * P]
                        else:
                            xt = xpool.tile([128, 4 * P], F32, tag="x")
                            load_blocks(xt, P, 0, src[g0 * 128 :, :], 0, P, g)
                            xg = xt[:, 0 : g * P]
                            # bf16 copy into the persistent tile (unscaled)
                            nc.vector.tensor_copy(
                                dst[:, (ci0 + g0) * P : (ci0 + g0 + g) * P], xg
                            )
                        for k in range(g):
                            ci = g0 + k
                            sq = sqpool.tile([128, P], BF16, tag="sq")
                            nc.scalar.square(sq[:], xg[:, k * P : (k + 1) * P])
                            for pb in range(2):
                                nc.tensor.matmul(
                                    ss[pb][:],
                                    ones_col[:],
                                    sq[:, pb * PB : (pb + 1) * PB],
                                    start=(ci == 0),
                                    stop=(ci == n_ci - 1),
                                )
                    for pb in range(2):
                        st = mini.tile([1, PB], F32, tag="st")
                        nc.scalar.sqrt(st[:], ss[pb][:])
                        si = mini.tile([1, PB], F32, tag="si")
                        nc.vector.reciprocal(si[:], st[:])
                        bc = pps.tile([128, PB], F32, tag=f"bc{pb}")
                        nc.tensor.matmul(bc[:], ones_row[:], si[:])
                        for ci in range(n_ci):
                            sl = slice(
                                (ci0 + ci) * P + pb * PB,
                                (ci0 + ci) * P + pb * PB + PB,
                            )
                            nc.vector.tensor_mul(dst[:, sl], dst[:, sl], bc[:])

            # ------------- main: support stream, logits, exp, vT -------------
            with ExitStack() as mctx:
                snpool = mctx.enter_context(tc.tile_pool(name="sn", bufs=2))
                snspool = mctx.enter_context(tc.tile_pool(name="sns", bufs=2))
                vpool = mctx.enter_context(tc.tile_pool(name="vs", bufs=2))
                msq = mctx.enter_context(tc.tile_pool(name="msq", bufs=2))
                mmini = mctx.enter_context(tc.tile_pool(name="mmini", bufs=2))
                lps = mctx.enter_context(
                    tc.tile_pool(name="logits", bufs=1, space="PSUM")
                )
                sps = mctx.enter_context(
                    tc.tile_pool(name="snps", bufs=1, space="PSUM")
                )
                vtps = mctx.enter_context(
                    tc.tile_pool(name="vtps", bufs=2, space="PSUM")
                )

                for qc in range(NQC):
                    qn = 128 if qc < NQC - 1 else QT
                    sn_sb = snpool.tile([128, NCI * 128], F32, tag="sn")
                    load_blocks(sn_sb, 128, 0, s4, qc * 128, qn, NC4)
                    load_blocks(sn_sb, 128, NC4, s3, qc * 128, qn, NC3)
                    v_sb = vpool.tile([128, NCV * 128], F32, tag="v")
                    load_blocks(v_sb, 128, 0, v, qc * 128, qn, NCV)

                    # support-side sums of squares (contract channel partitions)
                    ss4 = sps.tile([1, 128], F32, tag="ss4")
                    ss3 = sps.tile([1, 128], F32, tag="ss3")
                    for g0 in range(0, NCI, 4):
                        sq = msq.tile([128, 4 * 128], BF16, tag="sq")
                        if qn == 128:
                            nc.scalar.square(
                                sq[:], sn_sb[:, g0 * 128 : (g0 + 4) * 128]
                            )
                        else:
                            for k in range(4):
                                nc.scalar.square(
                                    sq[:, k * 128 : k * 128 + qn],
                                    sn_sb[:, (g0 + k) * 128 : (g0 + k) * 128 + qn],
                                )
                        for k in range(4):
                            ci = g0 + k
                            dst_ss = ss4 if ci < NC4 else ss3
                            first = ci == 0 or ci == NC4
                            last = ci == NC4 - 1 or ci == NCI - 1
                            nc.tensor.matmul(
                                dst_ss[:, 0:qn],
                                ones_col[:],
                                sq[:, k * 128 : k * 128 + qn],
                                start=first,
                                stop=last,
                            )
                    # per-support-pixel scales: TEMP*w_l / ||fs_l[:, q]||
                    bcs = []
                    for ln, ssp in ((0, ss4), (1, ss3)):
                        st = mmini.tile([1, 128], F32, tag=f"st{ln}")
                        nc.scalar.sqrt(st[:, 0:qn], ssp[:, 0:qn])
                        si = mmini.tile([1, 128], F32, tag=f"si{ln}")
                        nc.vector.reciprocal(si[:, 0:qn], st[:, 0:qn])
                        sw = mmini.tile([1, 128], F32, tag=f"sw{ln}")
                        nc.vector.tensor_scalar_mul(
                            sw[:, 0:qn], si[:, 0:qn], w_sb[0:1, ln : ln + 1]
                        )
                        bc = sps.tile([128, 128], F32, tag=f"bc{ln}")
                        nc.tensor.matmul(bc[:, 0:qn], ones_row[:], sw[:, 0:qn])
                        bcs.append(bc)
                    sn_s = snspool.tile([128, NCI * 128], BF16, tag="sns")
                    for ci in range(NCI):
                        bc = bcs[0] if ci < NC4 else bcs[1]
                        nc.vector.tensor_mul(
                            sn_s[:, ci * 128 : ci * 128 + qn],
                            sn_sb[:, ci * 128 : ci * 128 + qn],
                            bc[:, 0:qn],
                        )

                    # logits = sum_ci sns_ci.T @ qns_ci  -> [qn, PB] per block
                    for pb in range(2):
                        ps = lps.tile([128, PB], F32, tag=f"logits{pb}")
                        for ci in range(NCI):
                            nc.tensor.matmul(
                                ps[0:qn, :],
                                sn_s[:, ci * 128 : ci * 128 + qn],
                                qns[:, ci * P + pb * PB : ci * P + pb * PB + PB],
                                start=(ci == 0),
                                stop=(ci == NCI - 1),
                            )
                        nc.scalar.activation(
                            expT[0:qn, qc * P + pb * PB : qc * P + pb * PB + PB],
                            ps[0:qn, :],
                            AF.Exp,
                        )

                    # transpose f_s chunk: [c, q] -> [q, c] (bf16 on copy-out)
                    for cb in range(NCV):
                        tp = vtps.tile([128, 128], F32, tag="vt")
                        nc.tensor.transpose(
                            tp[0:qn, :], v_sb[:, cb * 128 : cb * 128 + qn], ident[:]
                        )
                        nc.scalar.copy(
                            vT[0:qn, qc * CV + cb * 128 : qc * CV + (cb + 1) * 128],
                            tp[0:qn, :],
                        )

            # ---------------- phase B: attention-weighted values ----------------
            with ExitStack() as bctx:
                yps = bctx.enter_context(
                    tc.tile_pool(name="yps", bufs=1, space="PSUM")
                )
                bps = bctx.enter_context(
                    tc.tile_pool(name="bps", bufs=1, space="PSUM")
                )
                bsq = bctx.enter_context(tc.tile_pool(name="bsq", bufs=2))
                bmini = bctx.enter_context(tc.tile_pool(name="bmini", bufs=2))
                bout = bctx.enter_context(tc.tile_pool(name="bout", bufs=2))

                for pb in range(2):
                    ys = [
                        yps.tile([128, PB], F32, tag=f"y{cb}", name=f"y{cb}") for cb in range(NCV)
                    ]
                    dn = bps.tile([1, PB], F32, tag="dn")
                    for qc in range(NQC):
                        esl = expT[:, qc * P + pb * PB : qc * P + pb * PB + PB]
                        for cb in range(NCV):
                            nc.tensor.matmul(
                                ys[cb][:],
                                vT[:, qc * CV + cb * 128 : qc * CV + (cb + 1) * 128],
                                esl,
                                start=(qc == 0),
                                stop=(qc == NQC - 1),
                            )
                        nc.tensor.matmul(
                            dn[:],
                            ones_col[:],
                            esl,
                            start=(qc == 0),
                            stop=(qc == NQC - 1),
                        )
                    # sum_c Y^2 for the att_fq l2norm (denominator cancels)
                    ssy = bps.tile([1, PB], F32, tag="ssy")
                    for cb in range(NCV):
                        sqy = bsq.tile([128, PB], BF16, tag="sqy")
                        nc.scalar.square(sqy[:], ys[cb][:])
                        nc.tensor.matmul(
                            ssy[:],
                            ones_col[:],
                            sqy[:],
                            start=(cb == 0),
                            stop=(cb == NCV - 1),
                        )
                    st = bmini.tile([1, PB], F32, tag="st")
                    # sqrt(ssy / ATT_WT^2) = ||Y|| / 0.3 ; reciprocal -> 0.3/||Y||
                    nc.scalar.activation(
                        st[:], ssy[:], AF.Sqrt, scale=float(1.0 / (ATT_WT * ATT_WT))
                    )
                    sv = bmini.tile([1, PB], F32, tag="sv")
                    nc.vector.reciprocal(sv[:], st[:])
                    dv = bmini.tile([1, PB], F32, tag="dv")
                    nc.vector.reciprocal(dv[:], dn[:])
                    bc_s = bps.tile([128, PB], F32, tag="bcs")
                    nc.tensor.matmul(bc_s[:], ones_row[:], sv[:])
                    bc_d = bps.tile([128, PB], F32, tag="bcd")
                    nc.tensor.matmul(bc_d[:], ones_row[:], dv[:])
                    bcs_sb = bout.tile([128, PB], F32, tag="bcs_sb")
                    nc.scalar.copy(bcs_sb[:], bc_s[:])
                    bcd_sb = bout.tile([128, PB], F32, tag="bcd_sb")
                    nc.scalar.copy(bcd_sb[:], bc_d[:])
                    for cb in range(NCV):
                        att_sb = bout.tile([128, PB], F32, tag="att")
                        nc.vector.tensor_mul(att_sb[:], ys[cb][:], bcd_sb[:])
                        nc.sync.dma_start(
                            att_o[cb * 128 : (cb + 1) * 128, pb * PB : (pb + 1) * PB],
                            att_sb[:],
                        )
                        t_sb = bout.tile([128, PB], F32, tag="t")
                        nc.vector.tensor_mul(t_sb[:], ys[cb][:], bcs_sb[:])
                        f_sb = bout.tile([128, PB], F32, tag="f")
                        nc.vector.tensor_add(
                            f_sb[:],
                            t_sb[:],
                            fqn[:, cb * P + pb * PB : cb * P + pb * PB + PB],
                        )
                        nc.sync.dma_start(
                            fq_o[cb * 128 : (cb + 1) * 128, pb * PB : (pb + 1) * PB],
                            f_sb[:],
                        )
    _split_sync_waits(nc)
    return nc


def make_in_maps(fq_l3, fs_l3, fq_l4, fs_l4, f_q, f_s, w_red):
    wvec = np.asarray(
        [[TEMP * float(w_red[0]), TEMP * float(w_red[1])]], dtype=np.float32
    )
    q4f = np.asarray(fq_l4, np.float32).reshape(B, C4, HW)
    q3f = np.asarray(fq_l3, np.float32).reshape(B, C3, HW)
    s4f = np.asarray(fs_l4, np.float32).reshape(B, C4, HW)
    s3f = np.asarray(fs_l3, np.float32).reshape(B, C3, HW)
    vf = np.asarray(f_s, np.float32).reshape(B, CV, HW)
    fqf = np.asarray(f_q, np.float32).reshape(B, CV, HW)
    in_maps = []
    for k in range(NCORES):
        b, j = divmod(k, PSH)
        sl = slice(j * P, (j + 1) * P)
        in_maps.append(
            {
                "q4": np.ascontiguousarray(q4f[b][:, sl]),
                "q3": np.ascontiguousarray(q3f[b][:, sl]),
                "s4": np.ascontiguousarray(s4f[b]),
                "s3": np.ascontiguousarray(s3f[b]),
                "v": np.ascontiguousarray(vf[b]),
                "fq": np.ascontiguousarray(fqf[b][:, sl]),
                "wv": wvec,
            }
        )
    return in_maps


def gather_outputs(results):
    att = np.empty((B, CV, HW), np.float32)
    fqo = np.empty((B, CV, HW), np.float32)
    for k in range(NCORES):
        b, j = divmod(k, PSH)
        sl = slice(j * P, (j + 1) * P)
        att[b][:, sl] = results[k]["att_o"]
        fqo[b][:, sl] = results[k]["fq_o"]
    return (
        fqo.reshape(B, CV, H, W),
        att.reshape(B, CV, H, W),
    )


def kernel(fq_l3, fs_l3, fq_l4, fs_l4, f_q, f_s, w_red, trace=False):
    nc = build()
    in_maps = make_in_maps(fq_l3, fs_l3, fq_l4, fs_l4, f_q, f_s, w_red)
    res = run_bass_kernel_spmd(nc, in_maps, core_ids=list(range(NCORES)), trace=trace)
    out = gather_outputs(res.results)
    if trace:
        return out, res
    return out
